# revision 15
# baseline (speedup 1.0000x reference)
# Trainium2 Bass kernel for nn_CrossAttentionLayer (linear attention with
# elu+1 feature map).
#
# Math (per batch n):
#   q = guidance @ Wq.T + bq ; k = x @ Wk.T + bk ; v = x @ Wv.T + bv
#   Q = elu(q)+1 ; K = elu(k)+1          (per head h, head dim D=64)
#   KV_h = K_h^T @ (v_h/S);  Z = 1/(Q_h . sum_s K_h + eps)
#   out_h = (Q_h @ KV_h) * Z * S         (the /S and *S cancel exactly)
#
# Sharding: 8 cores = batch(4) x guidance-halves(2). Each core recomputes
# K/V/KV/Ksum for its batch over the full source sequence S=4096 (dup x2),
# and the Q side for its 2048 guidance rows.
#
# On-chip dataflow (all matmuls in float32r: full-rate PE fp32):
#  phase 1 (per 128-row s-tile of x):
#    PE-transpose x-tile -> xT ; k/v projections token-major with xT as
#    stationary (bias added via a K=1 matmul against a ones row);
#    K = elu(k)+1 computed as max(relu(k+1), min(exp(k), 1));
#    KV accumulated in PSUM via 4 two-head matmuls (N=256); Ksum via a
#    ones-column matmul (N=512).
#  phase 2 (per 512-row l-chunk of guidance):
#    PE-transpose g -> gT ; q projection FEATURE-major (weights stationary,
#    per-partition bias via the activation); denominators via block-diagonal
#    Ksum matrix; out = (Q @ blockdiag(KV)) scaled by Z broadcast.

import sys

import numpy as np

if "/opt/trn_rl_repo" not in sys.path:
    sys.path.insert(0, "/opt/trn_rl_repo")

import concourse.bacc as bacc
import concourse.mybir as mybir
import concourse.tile as tile
from concourse import bass_utils
from concourse.masks import make_identity

P = 128
S = 4096
LC = 2048  # guidance rows per core
C = 512
H = 8
D = 64
NCT = C // P  # 4 column tiles
NST = S // P  # 32 s-tiles
EPS = 1e-6

F32 = mybir.dt.float32
F32R = mybir.dt.float32r

Exp = mybir.ActivationFunctionType.Exp
Relu = mybir.ActivationFunctionType.Relu


def _build_nc(reps=1):
    nc = bacc.Bacc(
        "TRN2",
        target_bir_lowering=False,
        debug=False,
        enable_asserts=False,
        num_devices=8,
    )
    xb = nc.dram_tensor("xb", [S, C], F32, kind="ExternalInput").ap()
    gb = nc.dram_tensor("gb", [LC, C], F32, kind="ExternalInput").ap()
    wkt = nc.dram_tensor("wkt", [C, C], F32, kind="ExternalInput").ap()
    wvt = nc.dram_tensor("wvt", [C, C], F32, kind="ExternalInput").ap()
    wqt = nc.dram_tensor("wqt", [C, C], F32, kind="ExternalInput").ap()
    bk = nc.dram_tensor("bk", [C], F32, kind="ExternalInput").ap()
    bv = nc.dram_tensor("bv", [C], F32, kind="ExternalInput").ap()
    bq = nc.dram_tensor("bq", [C], F32, kind="ExternalInput").ap()
    outb = nc.dram_tensor("outb", [LC, C], F32, kind="ExternalOutput").ap()

    with tile.TileContext(nc) as tc:
        for rep in range(reps):
            _emit(nc, tc, xb, gb, wkt, wvt, wqt, bk, bv, bq, outb, rep=rep)

    nc.compile()
    return nc


def _emit(nc, tc, xb, gb, wkt, wvt, wqt, bk, bv, bq, outb, rep=0):
    mm = nc.tensor.matmul
    with (
        tc.tile_pool(name=f"persist{rep}", bufs=1) as pp,
        tc.tile_pool(name=f"dram{rep}", bufs=1, space="DRAM") as dp,
    ):
        # --- constants / weights resident in SBUF ---
        # fp32r matmul operands must be produced by DVE/ACT compute ops (the
        # verifier requires an explicit rounding producer), so DMA/memset
        # results are staged in fp32 and copied into fp32r tiles on DVE.
        wk_sb = pp.tile([P, NCT, C], F32R)
        wv_sb = pp.tile([P, NCT, C], F32R)
        wq_sb = pp.tile([P, NCT, C], F32R)
        bk_row = pp.tile([1, C], F32R)
        bv_row = pp.tile([1, C], F32R)
        ones_row = pp.tile([1, P], F32R)
        with tc.tile_pool(name=f"init_{rep}", bufs=1) as ip:
            wk_st = ip.tile([P, NCT, C], F32, name="wk_st")
            wv_st = ip.tile([P, NCT, C], F32, name="wv_st")
            wq_st = ip.tile([P, NCT, C], F32, name="wq_st")
            nc.sync.dma_start(wk_st, wkt.rearrange("(t p) n -> p t n", p=P))
            nc.sync.dma_start(wv_st, wvt.rearrange("(t p) n -> p t n", p=P))
            nc.sync.dma_start(wq_st, wqt.rearrange("(t p) n -> p t n", p=P))
            nc.vector.tensor_copy(wk_sb, wk_st)
            nc.vector.tensor_copy(wv_sb, wv_st)
            nc.vector.tensor_copy(wq_sb, wq_st)
            bk_st = ip.tile([1, C], F32, name="bk_st")
            bv_st = ip.tile([1, C], F32, name="bv_st")
            nc.sync.dma_start(bk_st, bk.rearrange("(a c) -> a c", a=1))
            nc.sync.dma_start(bv_st, bv.rearrange("(a c) -> a c", a=1))
            nc.vector.tensor_copy(bk_row, bk_st)
            nc.vector.tensor_copy(bv_row, bv_st)
            ones_st = ip.tile([1, P], F32, name="ones_st")
            nc.vector.memset(ones_st, 1.0)
            nc.vector.tensor_copy(ones_row, ones_st)
        bqT = pp.tile([P, NCT], F32)
        nc.sync.dma_start(bqT, bq.rearrange("(t p) -> p t", p=P))
        bqT1 = pp.tile([P, NCT], F32)
        nc.vector.tensor_scalar_add(bqT1, bqT, 1.0)

        ident = pp.tile([P, P], F32)
        make_identity(nc, ident)
        zero_col = pp.tile([P, 1], F32)
        nc.vector.memset(zero_col, 0.0)
        ones_col = pp.tile([P, 1], F32)
        nc.vector.memset(ones_col, 1.0)

        # blockdiag(KV_h) as [cin_part, cin_tile, C] and blockdiag Ksum
        kvbd = pp.tile([P, NCT, C], F32R)
        nc.vector.tensor_copy(kvbd, zero_col[:, :, None].to_broadcast([P, NCT, C]))
        ksbd = pp.tile([P, NCT, H], F32R)
        nc.vector.tensor_copy(ksbd, zero_col[:, :, None].to_broadcast([P, NCT, H]))
        ksumT = pp.tile([P, NCT], F32)

        # ---------------- phase 1: x -> K,V -> KV, Ksum ----------------
        # Each KV accumulation group owns a full PSUM bank (start=True zeroes
        # the whole 2KB zero region). The V operand carries an extra ones
        # column so column 256 of each KV psum accumulates Ksum directly in
        # feature-major layout.
        with (
            tc.tile_pool(name=f"p1_{rep}", bufs=3) as p1,
            tc.tile_pool(name=f"p1ps_{rep}", bufs=2, space="PSUM") as p1ps,
            tc.tile_pool(name=f"tps_{rep}", bufs=2, space="PSUM") as tps,
            tc.tile_pool(name=f"accps_{rep}", bufs=1, space="PSUM") as accps,
        ):
            kv_ps = [
                accps.tile([P, 258], F32, tag=f"kv{hh}", name=f"kv_ps{hh}")
                for hh in range(4)
            ]
            for st in range(NST):
                first = st == 0
                last = st == NST - 1
                xt = p1.tile([P, C], F32, tag="xt")
                nc.sync.dma_start(xt, xb[st * P : (st + 1) * P, :])
                xT = p1.tile([P, NCT, P], F32R, tag="xT")
                for ci in range(NCT):
                    pt = tps.tile([P, P], F32, tag="tp")
                    nc.tensor.transpose(pt, xt[:, ci * P : (ci + 1) * P], ident)
                    nc.vector.tensor_copy(xT[:, ci, :], pt)
                # k projection (token-major): psum[s,cout]
                pk = p1ps.tile([P, C], F32, tag="proj")
                mm(pk, ones_row, bk_row, start=True, stop=False)
                for ci in range(NCT):
                    mm(pk, xT[:, ci, :], wk_sb[:, ci, :],
                       start=False, stop=(ci == NCT - 1))
                # v projection
                pv = p1ps.tile([P, C], F32, tag="proj")
                mm(pv, ones_row, bv_row, start=True, stop=False)
                for ci in range(NCT):
                    mm(pv, xT[:, ci, :], wv_sb[:, ci, :],
                       start=False, stop=(ci == NCT - 1))
                # V split into two 4-head halves, each with a ones column
                # (and a zero pad column: fp32r matmuls need an even free dim)
                v_ext = p1.tile([P, 2, 258], F32R, tag="v")
                nc.vector.tensor_copy(v_ext[:, 0, 0:256], pv[:, 0:256])
                nc.vector.tensor_copy(v_ext[:, 1, 0:256], pv[:, 256:512])
                nc.vector.tensor_copy(v_ext[:, 0, 256:257], ones_col)
                nc.vector.tensor_copy(v_ext[:, 1, 256:257], ones_col)
                nc.vector.tensor_copy(v_ext[:, 0, 257:258], zero_col)
                nc.vector.tensor_copy(v_ext[:, 1, 257:258], zero_col)
                # K = elu(k)+1 = max(relu(k+1), min(exp(k), 1))
                e_sb = p1.tile([P, C], F32, tag="e")
                nc.scalar.activation(e_sb, pk, Exp)
                u_sb = p1.tile([P, C], F32, tag="u")
                nc.scalar.activation(u_sb, pk, Relu, bias=1.0)
                nc.vector.tensor_scalar_min(e_sb, e_sb, 1.0)
                k_sb = p1.tile([P, C], F32R, tag="k")
                nc.vector.tensor_tensor(k_sb, e_sb, u_sb, mybir.AluOpType.max)
                # KV accumulation: two K-heads vs four V-heads (+ones) per mm
                for hh in range(4):
                    mm(kv_ps[hh],
                       k_sb[:, hh * P : (hh + 1) * P],
                       v_ext[:, hh // 2, :],
                       start=first, stop=last)

            # extract per-head KV blocks into blockdiag layout, and Ksum
            for h in range(H):
                hh = h // 2
                par = h % 2
                vcol = (h % 4) * D
                nc.vector.tensor_copy(
                    kvbd[par * D : (par + 1) * D, hh, h * D : (h + 1) * D],
                    kv_ps[hh][par * D : (par + 1) * D, vcol : vcol + D],
                )
            for hh in range(4):
                nc.vector.tensor_copy(ksumT[:, hh : hh + 1], kv_ps[hh][:, 256:257])
            # blockdiag Ksum [cin_part, cin_tile, H]
            for h in range(H):
                par = h % 2
                ct = h // 2
                nc.vector.tensor_copy(
                    ksbd[par * D : (par + 1) * D, ct, h : h + 1],
                    ksumT[par * D : (par + 1) * D, ct : ct + 1],
                )

        # ---------------- phase 2: guidance -> Q -> out ----------------
        with (
            tc.tile_pool(name=f"p2_{rep}", bufs=2) as p2,
            tc.tile_pool(name=f"p2ps_{rep}", bufs=2, space="PSUM") as p2ps,
            tc.tile_pool(name=f"tps2_{rep}", bufs=2, space="PSUM") as tps2,
            tc.tile_pool(name=f"dps_{rep}", bufs=2, space="PSUM") as dps,
        ):
            for lc in range(LC // C):
                gT = p2.tile([P, NCT, C], F32R, tag="gT")
                for lt in range(4):
                    gt = p2.tile([P, C], F32, tag="gt")
                    nc.sync.dma_start(
                        gt, gb[(lc * 4 + lt) * P : (lc * 4 + lt + 1) * P, :]
                    )
                    for ci in range(NCT):
                        pt = tps2.tile([P, P], F32, tag="tp2")
                        nc.tensor.transpose(pt, gt[:, ci * P : (ci + 1) * P], ident)
                        nc.vector.tensor_copy(gT[:, ci, lt * P : (lt + 1) * P], pt)
                # q projection, feature-major: psum[cout, l]
                qT = p2.tile([P, NCT, C], F32R, tag="qT")
                for ct in range(NCT):
                    pq = p2ps.tile([P, C], F32, tag="pq")
                    for ci in range(NCT):
                        mm(pq, wq_sb[:, ci, ct * P : (ct + 1) * P],
                           gT[:, ci, :], start=(ci == 0), stop=(ci == NCT - 1))
                    e2 = p2.tile([P, C], F32, tag="e2")
                    nc.scalar.activation(e2, pq, Exp, bias=bqT[:, ct : ct + 1])
                    u2 = p2.tile([P, C], F32, tag="u2")
                    nc.scalar.activation(u2, pq, Relu, bias=bqT1[:, ct : ct + 1])
                    nc.vector.tensor_scalar_min(e2, e2, 1.0)
                    nc.vector.tensor_tensor(
                        qT[:, ct, :], e2, u2, mybir.AluOpType.max
                    )
                # per 128-row l-tile: denominators, then output
                for lt in range(4):
                    lsl = slice(lt * P, (lt + 1) * P)
                    pd = dps.tile([P, H], F32, tag="pd")
                    for ct in range(NCT):
                        mm(pd, qT[:, ct, lsl], ksbd[:, ct, :],
                           start=(ct == 0), stop=(ct == NCT - 1))
                    zl = p2.tile([P, H], F32, tag="zl")
                    nc.vector.tensor_scalar_add(zl, pd, EPS)
                    nc.vector.reciprocal(zl, zl)
                    po = p2ps.tile([P, C], F32, tag="po")
                    for ct in range(NCT):
                        mm(po, qT[:, ct, lsl], kvbd[:, ct, :],
                           start=(ct == 0), stop=(ct == NCT - 1))
                    osb = p2.tile([P, C], F32, tag="osb")
                    nc.vector.tensor_tensor(
                        osb.rearrange("p (h v) -> p h v", h=H),
                        po.rearrange("p (h v) -> p h v", h=H),
                        zl[:, :, None].to_broadcast([P, H, D]),
                        mybir.AluOpType.mult,
                    )
                    nc.sync.dma_start(
                        outb[(lc * 4 + lt) * P : (lc * 4 + lt + 1) * P, :], osb
                    )


_CACHE = {}


def _get_nc(reps=1):
    key = ("nc", reps)
    if key not in _CACHE:
        _CACHE[key] = _build_nc(reps)
    return _CACHE[key]


def _make_runner(nc):
    """Build a reusable jitted SPMD runner for `nc` (mirrors
    bass2jax.run_bass_via_pjrt's multi-core branch, but caches the jit so
    repeated calls don't re-lower/re-compile)."""
    import jax
    from jax.sharding import Mesh, PartitionSpec
    from jax.experimental.shard_map import shard_map

    import concourse.mybir as mb
    from concourse import bass2jax

    bass2jax.install_neuronx_cc_hook()

    n_cores = 8
    partition_name = (
        nc.partition_id_tensor.name if nc.partition_id_tensor else None
    )
    in_names, out_names, out_avals, zero_shapes = [], [], [], []
    for alloc in nc.m.functions[0].allocations:
        if not isinstance(alloc, mb.MemoryLocationSet):
            continue
        name = alloc.memorylocations[0].name
        if alloc.kind == "ExternalInput":
            if name != partition_name:
                in_names.append(name)
        elif alloc.kind == "ExternalOutput":
            shape = tuple(alloc.tensor_shape)
            dtype = mb.dt.np(alloc.dtype)
            out_names.append(name)
            out_avals.append(jax.core.ShapedArray(shape, dtype))
            zero_shapes.append((shape, dtype))
    n_params = len(in_names)
    n_outs = len(out_names)
    all_names = in_names + out_names
    if partition_name is not None:
        all_names.append(partition_name)
    donate = tuple(range(n_params, n_params + n_outs))

    def _body(*args):
        operands = list(args)
        if partition_name is not None:
            operands.append(bass2jax.partition_id_tensor())
        outs = bass2jax._bass_exec_p.bind(
            *operands,
            out_avals=tuple(out_avals),
            in_names=tuple(all_names),
            out_names=tuple(out_names),
            lowering_input_output_aliases=(),
            sim_require_finite=True,
            sim_require_nnan=True,
            nc=nc,
        )
        return tuple(outs)

    devices = jax.devices()[:n_cores]
    mesh = Mesh(np.asarray(devices), ("core",))
    in_specs = (PartitionSpec("core"),) * (n_params + n_outs)
    out_specs = (PartitionSpec("core"),) * n_outs
    sharded = jax.jit(
        shard_map(
            _body, mesh=mesh, in_specs=in_specs, out_specs=out_specs,
            check_rep=False,
        ),
        donate_argnums=donate,
        keep_unused=True,
    )

    def _zeros():
        return [
            np.zeros((n_cores * sh[0], *sh[1:]), dt) for sh, dt in zero_shapes
        ]

    def runner(concat_in):
        out_arrs = sharded(*concat_in, *_zeros())
        return [
            {
                name: np.asarray(out_arrs[i]).reshape(
                    n_cores, *out_avals[i].shape
                )[c]
                for i, name in enumerate(out_names)
            }
            for c in range(n_cores)
        ]

    def concat(maps):
        return [
            np.concatenate([np.asarray(m[name]) for m in maps], axis=0)
            for name in in_names
        ]

    def timed(concat_in, n=10, warmup=2):
        """Time `n` executions with device-resident inputs and on-device
        donated zero outputs, so per-call host traffic is ~zero."""
        import time as _time
        import jax.numpy as jnp
        from jax.sharding import NamedSharding

        sh = NamedSharding(mesh, PartitionSpec("core"))
        dev_in = [jax.device_put(a, sh) for a in concat_in]

        @jax.jit
        def _mkzeros():
            return tuple(
                jnp.zeros((n_cores * s[0], *s[1:]), d) for s, d in zero_shapes
            )

        _mkzeros = jax.jit(_mkzeros, out_shardings=(sh,) * n_outs)
        times = []
        for i in range(warmup + n):
            z = jax.block_until_ready(_mkzeros())
            t0 = _time.perf_counter()
            outs = sharded(*dev_in, *z)
            jax.block_until_ready(outs)
            dt = _time.perf_counter() - t0
            if i >= warmup:
                times.append(dt)
        return times

    return runner, concat, timed


def _in_maps(x, guidance, Wq, bq, Wk, bk, Wv, bv):
    x = np.ascontiguousarray(x, dtype=np.float32)
    guidance = np.ascontiguousarray(guidance, dtype=np.float32)
    wqt = np.ascontiguousarray(np.asarray(Wq, dtype=np.float32).T)
    wkt = np.ascontiguousarray(np.asarray(Wk, dtype=np.float32).T)
    wvt = np.ascontiguousarray(np.asarray(Wv, dtype=np.float32).T)
    bq = np.ascontiguousarray(bq, dtype=np.float32)
    bk = np.ascontiguousarray(bk, dtype=np.float32)
    bv = np.ascontiguousarray(bv, dtype=np.float32)
    maps = []
    for core in range(8):
        b, half = core // 2, core % 2
        maps.append(
            {
                "xb": np.ascontiguousarray(x[b]),
                "gb": np.ascontiguousarray(guidance[b, half * LC : (half + 1) * LC]),
                "wqt": wqt,
                "wkt": wkt,
                "wvt": wvt,
                "bq": bq,
                "bk": bk,
                "bv": bv,
            }
        )
    return maps


def _gather(results):
    B = 4
    out = np.empty((B, 2 * LC, C), dtype=np.float32)
    for core in range(8):
        b, half = core // 2, core % 2
        out[b, half * LC : (half + 1) * LC] = results[core]["outb"]
    return out


def run(inputs, reps=1):
    nc = _get_nc(reps)
    key = ("runner", reps)
    if key not in _CACHE:
        _CACHE[key] = _make_runner(nc)
    runner, concat, timed = _CACHE[key]
    maps = _in_maps(**inputs)
    return runner, timed, concat(maps)


def kernel(**inputs):
    runner, _, concat_in = run(inputs)
    return _gather(runner(concat_in))
